# revision 18
# baseline (speedup 1.0000x reference)
"""Masked dot-product attention on 8 Trainium2 NeuronCores (Bass/Tile).

Problem: B=16, LQ=LK=2048, D=128 fp32; per-batch key valid_lens mask.
Sharding: 64 (batch, 512-query-block) units bin-packed into 8 slots x 8
cores by per-batch valid k-tile count, so every core runs an identical
(SPMD) program while skipping masked-out key tiles entirely.

Per unit, on device (scores kept transposed so no probability transpose
is ever needed):
  S^T[k, q]  = KT_tile.T @ QT_block       (fp32r matmuls, N=512, full rate)
  W^T        = exp(S^T/sqrt(D) + bias[k]) (ScalarE; bias -1e4 masks invalid
                                           key rows -> exp underflows to 0)
  rowsum[q] += ones.T @ W^T               (PE, M=1, bf16, PSUM accumulate)
  O^T[d, q] += V_tile.T @ W^T             (PE, bf16, PSUM accumulate)
then O^T * (1/rowsum) (GPSIMD partition-broadcast + DVE), PE-transpose
back to [q, d], evict via ScalarE, DMA out. bf16 is used for the
probability/value matmuls because fp32r accumulating matmuls
(start=False) measured ~15x slower on hardware.
"""

import math

import ml_dtypes
import numpy as np

import concourse.bass as bass
import concourse.mybir as mybir
import concourse.tile as tile
from concourse import bacc
from concourse.bass_utils import run_bass_kernel_spmd
from concourse.masks import make_identity

B, LQ, LK, D = 16, 2048, 2048, 128
N_CORES = 8
QB = 512          # query block (one unit) = QB rows of Q
N_SLOTS = (LQ // QB) * B // N_CORES   # 8 slots per core
KT = 128          # key tile
F32 = mybir.dt.float32
F32R = mybir.dt.float32r
BF16 = mybir.dt.bfloat16
FP16 = mybir.dt.float16
SCALE = 1.0 / math.sqrt(D)


def _plan(valid_lens):
    """Assign 64 (batch, qblock) units to an 8x8 (slot, core) grid.

    Returns (slot_units, slot_ntiles, masked_from) where
      slot_units[s][c] = (batch, qblock) handled by core c in slot s
      slot_ntiles[s]   = k-tiles processed in slot s (max over cores)
      masked_from[s]   = first k-tile index needing a mask multiply
    """
    vl = np.asarray(valid_lens).astype(np.int64)
    ktiles = np.maximum(1, np.ceil(vl / KT).astype(np.int64))
    units = [(int(b), j) for b in range(B) for j in range(LQ // QB)]
    units.sort(key=lambda u: -ktiles[u[0]])
    slot_units, slot_ntiles, masked_from = [], [], []
    for s in range(N_SLOTS):
        chunk = units[s * N_CORES:(s + 1) * N_CORES]
        slot_units.append(chunk)
        slot_ntiles.append(int(max(ktiles[b] for b, _ in chunk)))
        # tile t is fully valid for all cores iff t < min(floor(vl/KT));
        # from there on a multiplicative mask is required on some core.
        masked_from.append(int(min(vl[b] // KT for b, _ in chunk)))
    return slot_units, slot_ntiles, masked_from


def _pack_inputs(queries, keys, values, valid_lens, slot_units, slot_ntiles):
    """Build per-core packed input arrays (host-side numpy)."""
    vl = np.asarray(valid_lens).astype(np.int64)
    qt = np.ascontiguousarray(np.transpose(queries, (0, 2, 1)))   # [B, D, LQ]
    kt = np.ascontiguousarray(np.transpose(keys, (0, 2, 1)))      # [B, D, LK]
    u_total = sum(slot_ntiles)
    in_maps = []
    for c in range(N_CORES):
        qt_p = np.zeros((N_SLOTS, D, QB), np.float16)
        kt_p = np.zeros((u_total, D, KT), np.float16)
        v_p = np.zeros((u_total, KT, D), np.float16)
        mask_p = np.zeros((KT, u_total), np.float32)
        off = 0
        for s in range(N_SLOTS):
            b, j = slot_units[s][c]
            qt_p[s] = qt[b, :, j * QB:(j + 1) * QB]
            n_valid = int(min(slot_ntiles[s], math.ceil(vl[b] / KT)))
            for t in range(n_valid):
                k0 = t * KT
                kt_p[off + t] = kt[b, :, k0:k0 + KT]
                v_p[off + t] = values[b, k0:k0 + KT, :]
                nv = int(min(KT, vl[b] - k0))
                mask_p[:nv, off + t] = 1.0
            off += slot_ntiles[s]
        bias_p = (mask_p - 1.0) * 10000.0
        in_maps.append({
            "qt": qt_p, "kt": kt_p, "v": v_p, "mask": bias_p,
            "ones": np.ones((KT, 1), np.float16),
        })
    return in_maps, u_total


def build_kernel(slot_ntiles, masked_from, u_total, reps=1,
                 do_exp=True, do_rs=True, do_pv=True, do_epi=True):
    nc = bacc.Bacc(None, target_bir_lowering=False, debug=True)
    qt_d = nc.dram_tensor("qt", [N_SLOTS, D, QB], FP16, kind="ExternalInput")
    kt_d = nc.dram_tensor("kt", [u_total, D, KT], FP16, kind="ExternalInput")
    v_d = nc.dram_tensor("v", [u_total, KT, D], FP16, kind="ExternalInput")
    mask_d = nc.dram_tensor("mask", [KT, u_total], F32, kind="ExternalInput")
    ones_d = nc.dram_tensor("ones", [KT, 1], FP16, kind="ExternalInput")
    out_d = nc.dram_tensor("out", [N_SLOTS, QB, D], F32, kind="ExternalOutput")

    G = 2  # k-tiles per exp group (PSUM banks: st 2x2 + ot 2 + rs 1 + to 1 = 8)

    with tile.TileContext(nc) as tc:
        with (
            tc.tile_pool(name="const", bufs=1) as const,
            tc.tile_pool(name="wt_pool", bufs=4) as wt_pool,
            tc.tile_pool(name="onorm_pool", bufs=2) as onorm_pool,
            tc.tile_pool(name="ostage_pool", bufs=2) as ostage_pool,
            tc.tile_pool(name="recip_pool", bufs=2) as recip_pool,
            tc.tile_pool(name="bc_pool", bufs=2) as bc_pool,
            tc.tile_pool(name="st_psum", bufs=2, space="PSUM") as st_psum,
            tc.tile_pool(name="ot_psum", bufs=2, space="PSUM") as ot_psum,
            tc.tile_pool(name="rs_psum", bufs=1, space="PSUM") as rs_psum,
            tc.tile_pool(name="to_psum", bufs=1, space="PSUM") as to_psum,
        ):
            identity = const.tile([128, 128], FP16)
            make_identity(nc, identity)
            ones = const.tile([128, 1], FP16)
            nc.sync.dma_start(out=ones, in_=ones_d[:, :])
            qt_all = const.tile([128, N_SLOTS, QB], FP16)
            nc.sync.dma_start(
                out=qt_all,
                in_=qt_d.rearrange("s d q -> d s q"),
            )
            mask_all = const.tile([128, u_total], F32)
            nc.sync.dma_start(out=mask_all, in_=mask_d[:, :])
            kt_all = const.tile([128, u_total, KT], FP16)
            v_all = const.tile([128, u_total, D], FP16)
            nchunk = 4
            bnds = [round(i * u_total / nchunk) for i in range(nchunk + 1)]
            for lo, hi in zip(bnds[:-1], bnds[1:]):
                nc.sync.dma_start(
                    out=kt_all[:, lo:hi, :],
                    in_=kt_d[lo:hi].rearrange("u d k -> d u k"))
                nc.sync.dma_start(
                    out=v_all[:, lo:hi, :],
                    in_=v_d[lo:hi].rearrange("u k d -> k u d"))

            # reps>1 repeats computation for timing isolation (same output)
            for _rep in range(reps):
                off = 0
                for s in range(len(slot_ntiles)):
                    u_s = slot_ntiles[s]
                    ot = ot_psum.tile([128, QB], F32)          # O^T accum [d, q]
                    rs = rs_psum.tile([1, QB], F32)            # rowsum [1, q]
                    qt_s = qt_all[:, s, :]
                    # merged exps below masked_from; per-tile exps with an
                    # additive mask bias (-1e4 -> exp underflows to 0) after
                    groups, t0 = [], 0
                    while t0 < u_s:
                        if t0 < masked_from[s]:
                            gsz = min(G, masked_from[s] - t0, u_s - t0)
                        else:
                            gsz = 1
                        groups.append((t0, gsz))
                        t0 += gsz
                    for g, gsz in groups:
                        st = st_psum.tile([128, G, QB], F32)   # S^T group
                        wt = wt_pool.tile([128, G, QB], FP16)   # exp(S^T) group
                        for tt in range(gsz):
                            t = g + tt
                            nc.tensor.matmul(
                                st[:, tt, :], kt_all[:, off + t, :], qt_s,
                                start=True, stop=True,
                            )
                        bias = (0.0 if g < masked_from[s]
                                else mask_all[:, off + g:off + g + 1])
                        nc.scalar.activation(
                            wt[:, :gsz, :], st[:, :gsz, :],
                            mybir.ActivationFunctionType.Exp, scale=SCALE,
                            bias=bias,
                        )
                        for tt in range(gsz):
                            t = g + tt
                            wt_t = wt[:, tt, :]
                            if do_rs:
                                nc.tensor.matmul(
                                    rs, ones, wt_t,
                                    start=(t == 0), stop=(t == u_s - 1),
                                )
                            if do_pv == "noaccum":
                                nc.tensor.matmul(
                                    ot, v_all[:, off + t, :], wt_t,
                                    start=True, stop=True,
                                )
                            elif do_pv:
                                nc.tensor.matmul(
                                    ot, v_all[:, off + t, :], wt_t,
                                    start=(t == 0), stop=(t == u_s - 1),
                                )
                    off += u_s
                    if not do_epi:
                        continue

                    recip = recip_pool.tile([1, QB], F32)
                    nc.vector.reciprocal(recip, rs)
                    recip_bc = bc_pool.tile([128, QB], F32)
                    nc.gpsimd.partition_broadcast(recip_bc, recip)
                    onorm = onorm_pool.tile([128, QB], FP16)
                    nc.vector.tensor_mul(onorm, ot, recip_bc)
                    to = to_psum.tile([128, QB], FP16)
                    for j in range(4):
                        nc.tensor.transpose(
                            to[:, j * 128:(j + 1) * 128],
                            onorm[:, j * 128:(j + 1) * 128],
                            identity,
                        )
                    ostage = ostage_pool.tile([128, 4, 128], F32)
                    nc.vector.tensor_copy(ostage, to)
                    nc.sync.dma_start(
                        out=out_d[s].rearrange("(sub p) d -> p sub d", p=128),
                        in_=ostage,
                    )
    nc.finalize()
    return nc


def kernel(queries, keys, values, valid_lens):
    queries = np.ascontiguousarray(np.asarray(queries, dtype=np.float32))
    keys = np.ascontiguousarray(np.asarray(keys, dtype=np.float32))
    values = np.ascontiguousarray(np.asarray(values, dtype=np.float32))
    assert queries.shape == (B, LQ, D), queries.shape
    assert keys.shape == (B, LK, D), keys.shape
    assert values.shape == (B, LK, D), values.shape

    slot_units, slot_ntiles, masked_from = _plan(valid_lens)
    in_maps, u_total = _pack_inputs(
        queries, keys, values, valid_lens, slot_units, slot_ntiles)
    nc = build_kernel(slot_ntiles, masked_from, u_total)
    res = None
    last_exc = None
    for attempt in range(3):
        try:
            res = run_bass_kernel_spmd(nc, in_maps, list(range(N_CORES)))
            break
        except Exception as exc:  # transient NRT/axon failures
            last_exc = exc
            try:
                import jax
                jax.clear_caches()
            except Exception:
                pass
    if res is None:
        raise last_exc

    out = np.empty((B, LQ, D), np.float32)
    for c in range(N_CORES):
        o = res.results[c]["out"]
        for s in range(N_SLOTS):
            b, j = slot_units[s][c]
            out[b, j * QB:(j + 1) * QB, :] = o[s]
    return out



# revision 23
# speedup vs baseline: 1.0617x; 1.0617x over previous
"""Masked dot-product attention on 8 Trainium2 NeuronCores (Bass/Tile).

Problem: B=16, LQ=LK=2048, D=128 fp32; per-batch key valid_lens mask.
Sharding: 64 (batch, 512-query-block) units bin-packed into 8 slots x 8
cores by per-batch valid k-tile count, so every core runs an identical
(SPMD) program while skipping masked-out key tiles entirely.

Per unit, on device (scores kept transposed so no probability transpose
is ever needed):
  S^T[k, q]  = KT_tile.T @ QT_block       (fp32r matmuls, N=512, full rate)
  W^T        = exp(S^T/sqrt(D) + bias[k]) (ScalarE; bias -1e4 masks invalid
                                           key rows -> exp underflows to 0)
  rowsum[q] += ones.T @ W^T               (PE, M=1, bf16, PSUM accumulate)
  O^T[d, q] += V_tile.T @ W^T             (PE, bf16, PSUM accumulate)
then O^T * (1/rowsum) (GPSIMD partition-broadcast + DVE), PE-transpose
back to [q, d], evict via ScalarE, DMA out. bf16 is used for the
probability/value matmuls because fp32r accumulating matmuls
(start=False) measured ~15x slower on hardware.
"""

import math

import ml_dtypes
import numpy as np

import concourse.bass as bass
import concourse.mybir as mybir
import concourse.tile as tile
from concourse import bacc
from concourse.bass_utils import run_bass_kernel_spmd
from concourse.masks import make_identity

B, LQ, LK, D = 16, 2048, 2048, 128
N_CORES = 8
QB = 512          # query block (one unit) = QB rows of Q
N_SLOTS = (LQ // QB) * B // N_CORES   # 8 slots per core
KT = 128          # key tile
F32 = mybir.dt.float32
F32R = mybir.dt.float32r
BF16 = mybir.dt.bfloat16
FP16 = mybir.dt.float16
SCALE = 1.0 / math.sqrt(D)


def _plan(valid_lens):
    """Assign 64 (batch, qblock) units to an 8x8 (slot, core) grid.

    Returns (slot_units, slot_ntiles, masked_from) where
      slot_units[s][c] = (batch, qblock) handled by core c in slot s
      slot_ntiles[s]   = k-tiles processed in slot s (max over cores)
      masked_from[s]   = first k-tile index needing a mask multiply
    """
    vl = np.asarray(valid_lens).astype(np.int64)
    ktiles = np.maximum(1, np.ceil(vl / KT).astype(np.int64))
    units = [(int(b), j) for b in range(B) for j in range(LQ // QB)]
    units.sort(key=lambda u: -ktiles[u[0]])
    slot_units, slot_ntiles, masked_from = [], [], []
    for s in range(N_SLOTS):
        chunk = units[s * N_CORES:(s + 1) * N_CORES]
        slot_units.append(chunk)
        slot_ntiles.append(int(max(ktiles[b] for b, _ in chunk)))
        # tile t is fully valid for all cores iff t < min(floor(vl/KT));
        # from there on a multiplicative mask is required on some core.
        masked_from.append(int(min(vl[b] // KT for b, _ in chunk)))
    return slot_units, slot_ntiles, masked_from


def _pack_inputs(queries, keys, values, valid_lens, slot_units, slot_ntiles):
    """Build per-core packed input arrays (host-side numpy)."""
    vl = np.asarray(valid_lens).astype(np.int64)
    qt = np.ascontiguousarray(np.transpose(queries, (0, 2, 1)))   # [B, D, LQ]
    kt = np.ascontiguousarray(np.transpose(keys, (0, 2, 1)))      # [B, D, LK]
    u_total = sum(slot_ntiles)
    in_maps = []
    for c in range(N_CORES):
        qt_p = np.zeros((N_SLOTS, D, QB), np.float16)
        kt_p = np.zeros((u_total, D, KT), np.float16)
        v_p = np.zeros((u_total, KT, D), np.float16)
        mask_p = np.zeros((KT, u_total), np.float32)
        off = 0
        for s in range(N_SLOTS):
            b, j = slot_units[s][c]
            qt_p[s] = qt[b, :, j * QB:(j + 1) * QB]
            n_valid = int(min(slot_ntiles[s], math.ceil(vl[b] / KT)))
            for t in range(n_valid):
                k0 = t * KT
                kt_p[off + t] = kt[b, :, k0:k0 + KT]
                v_p[off + t] = values[b, k0:k0 + KT, :]
                nv = int(min(KT, vl[b] - k0))
                mask_p[:nv, off + t] = 1.0
            off += slot_ntiles[s]
        bias_p = (mask_p - 1.0) * 10000.0
        in_maps.append({
            "qt": qt_p, "kt": kt_p, "v": v_p, "mask": bias_p,
            "ones": np.ones((KT, 1), np.float16),
        })
    return in_maps, u_total


def build_kernel(slot_ntiles, masked_from, u_total, reps=1,
                 do_exp=True, do_rs=True, do_pv=True, do_epi=True):
    nc = bacc.Bacc(None, target_bir_lowering=False, debug=True)
    qt_d = nc.dram_tensor("qt", [N_SLOTS, D, QB], FP16, kind="ExternalInput")
    kt_d = nc.dram_tensor("kt", [u_total, D, KT], FP16, kind="ExternalInput")
    v_d = nc.dram_tensor("v", [u_total, KT, D], FP16, kind="ExternalInput")
    mask_d = nc.dram_tensor("mask", [KT, u_total], F32, kind="ExternalInput")
    ones_d = nc.dram_tensor("ones", [KT, 1], FP16, kind="ExternalInput")
    out_d = nc.dram_tensor("out", [N_SLOTS, QB, D], F32, kind="ExternalOutput")

    G = 2  # k-tiles per exp group (PSUM banks: st 2x2 + ot 2 + rs 1 + to 1 = 8)

    with tile.TileContext(nc) as tc:
        with (
            tc.tile_pool(name="const", bufs=1) as const,
            tc.tile_pool(name="wt_pool", bufs=4) as wt_pool,
            tc.tile_pool(name="ws_pool", bufs=3) as ws_pool,
            tc.tile_pool(name="onorm_pool", bufs=2) as onorm_pool,
            tc.tile_pool(name="ostage_pool", bufs=2) as ostage_pool,
            tc.tile_pool(name="recip_pool", bufs=2) as recip_pool,
            tc.tile_pool(name="bc_pool", bufs=2) as bc_pool,
            tc.tile_pool(name="st_psum", bufs=2, space="PSUM") as st_psum,
            tc.tile_pool(name="ot_psum", bufs=2, space="PSUM") as ot_psum,
            tc.tile_pool(name="rs_psum", bufs=1, space="PSUM") as rs_psum,
            tc.tile_pool(name="to_psum", bufs=1, space="PSUM") as to_psum,
        ):
            identity = const.tile([128, 128], FP16)
            make_identity(nc, identity)
            ones = const.tile([128, 1], FP16)
            nc.sync.dma_start(out=ones, in_=ones_d[:, :])
            qt_all = const.tile([128, N_SLOTS, QB], FP16)
            for s0 in range(0, N_SLOTS, 2):
                nc.sync.dma_start(
                    out=qt_all[:, s0:s0 + 2, :],
                    in_=qt_d[s0:s0 + 2].rearrange("s d q -> d s q"),
                )
            mask_all = const.tile([128, u_total], F32)
            nc.sync.dma_start(out=mask_all, in_=mask_d[:, :])
            kt_all = const.tile([128, u_total, KT], FP16)
            v_all = const.tile([128, u_total, D], FP16)
            bnds = [0, 2, 6] + [
                round(6 + i * (u_total - 6) / 6) for i in range(1, 7)]
            for lo, hi in zip(bnds[:-1], bnds[1:]):
                nc.sync.dma_start(
                    out=kt_all[:, lo:hi, :],
                    in_=kt_d[lo:hi].rearrange("u d k -> d u k"))
                nc.sync.dma_start(
                    out=v_all[:, lo:hi, :],
                    in_=v_d[lo:hi].rearrange("u k d -> k u d"))

            # reps>1 repeats computation for timing isolation (same output)
            pending_epi = []
            for _rep in range(reps):
                off = 0
                for s in range(len(slot_ntiles)):
                    u_s = slot_ntiles[s]
                    ot = ot_psum.tile([128, QB], F32)          # O^T accum [d, q]
                    rs = rs_psum.tile([1, QB], F32)            # rowsum [1, q]
                    qt_s = qt_all[:, s, :]
                    # merged exps below masked_from; per-tile exps with an
                    # additive mask bias (-1e4 -> exp underflows to 0) after
                    groups, t0 = [], 0
                    while t0 < u_s:
                        if t0 < masked_from[s]:
                            gsz = min(G, masked_from[s] - t0, u_s - t0)
                        else:
                            gsz = 1
                        groups.append((t0, gsz))
                        t0 += gsz
                    for g, gsz in groups:
                        st = st_psum.tile([128, G, QB], F32)   # S^T group
                        wt = wt_pool.tile([128, G, QB], FP16)   # exp(S^T) group
                        for tt in range(gsz):
                            t = g + tt
                            nc.tensor.matmul(
                                st[:, tt, :], kt_all[:, off + t, :], qt_s,
                                start=True, stop=True,
                            )
                        bias = (0.0 if g < masked_from[s]
                                else mask_all[:, off + g:off + g + 1])
                        nc.scalar.activation(
                            wt[:, :gsz, :], st[:, :gsz, :],
                            mybir.ActivationFunctionType.Exp, scale=SCALE,
                            bias=bias,
                        )
                        if g == 0 and pending_epi:
                            # previous slot's epilogue goes here, behind this
                            # slot's first QK group, so the in-order PE queue
                            # never stalls on the normalize chain
                            pending_epi.pop(0)()
                        if do_rs:
                            # rowsum is linear in the k-tiles: pre-sum pairs
                            # on DVE so PE streams half the rowsum matmuls
                            if gsz == 2:
                                ws = ws_pool.tile([128, QB], FP16)
                                nc.vector.tensor_add(
                                    ws, wt[:, 0, :], wt[:, 1, :])
                                rs_src = ws
                            else:
                                rs_src = wt[:, 0, :]
                            nc.tensor.matmul(
                                rs, ones, rs_src,
                                start=(g == 0), stop=(g + gsz == u_s),
                            )
                        for tt in range(gsz):
                            t = g + tt
                            wt_t = wt[:, tt, :]
                            if do_pv == "noaccum":
                                nc.tensor.matmul(
                                    ot, v_all[:, off + t, :], wt_t,
                                    start=True, stop=True,
                                )
                            elif do_pv:
                                nc.tensor.matmul(
                                    ot, v_all[:, off + t, :], wt_t,
                                    start=(t == 0), stop=(t == u_s - 1),
                                )
                    off += u_s
                    if not do_epi:
                        continue

                    def _epilogue(s=s, ot=ot, rs=rs):
                        recip = recip_pool.tile([1, QB], F32)
                        nc.vector.reciprocal(recip, rs)
                        recip_bc = bc_pool.tile([128, QB], F32)
                        nc.gpsimd.partition_broadcast(recip_bc, recip)
                        onorm = onorm_pool.tile([128, QB], FP16)
                        nc.vector.tensor_mul(onorm, ot, recip_bc)
                        to = to_psum.tile([128, QB], FP16)
                        for j in range(4):
                            nc.tensor.transpose(
                                to[:, j * 128:(j + 1) * 128],
                                onorm[:, j * 128:(j + 1) * 128],
                                identity,
                            )
                        ostage = ostage_pool.tile([128, 4, 128], F32)
                        nc.vector.tensor_copy(ostage, to)
                        nc.sync.dma_start(
                            out=out_d[s].rearrange(
                                "(sub p) d -> p sub d", p=128),
                            in_=ostage,
                        )
                    pending_epi.append(_epilogue)
            for fn in pending_epi:
                fn()
    nc.finalize()
    return nc


def kernel(queries, keys, values, valid_lens):
    queries = np.ascontiguousarray(np.asarray(queries, dtype=np.float32))
    keys = np.ascontiguousarray(np.asarray(keys, dtype=np.float32))
    values = np.ascontiguousarray(np.asarray(values, dtype=np.float32))
    assert queries.shape == (B, LQ, D), queries.shape
    assert keys.shape == (B, LK, D), keys.shape
    assert values.shape == (B, LK, D), values.shape

    slot_units, slot_ntiles, masked_from = _plan(valid_lens)
    in_maps, u_total = _pack_inputs(
        queries, keys, values, valid_lens, slot_units, slot_ntiles)
    nc = build_kernel(slot_ntiles, masked_from, u_total)
    res = None
    last_exc = None
    for attempt in range(3):
        try:
            res = run_bass_kernel_spmd(nc, in_maps, list(range(N_CORES)))
            break
        except Exception as exc:  # transient NRT/axon failures
            last_exc = exc
            try:
                import jax
                jax.clear_caches()
            except Exception:
                pass
    if res is None:
        raise last_exc

    out = np.empty((B, LQ, D), np.float32)
    for c in range(N_CORES):
        o = res.results[c]["out"]
        for s in range(N_SLOTS):
            b, j = slot_units[s][c]
            out[b, j * QB:(j + 1) * QB, :] = o[s]
    return out



# revision 24
# speedup vs baseline: 1.1769x; 1.1084x over previous
"""Masked dot-product attention on 8 Trainium2 NeuronCores (Bass/Tile).

Problem: B=16, LQ=LK=2048, D=128 fp32; per-batch key valid_lens mask.
Sharding: 64 (batch, 512-query-block) units bin-packed into 8 slots x 8
cores by per-batch valid k-tile count, so every core runs an identical
(SPMD) program while skipping masked-out key tiles entirely.

Per unit, on device (scores kept transposed so no probability transpose
is ever needed):
  S^T[k, q]  = KT_tile.T @ QT_block       (fp32r matmuls, N=512, full rate)
  W^T        = exp(S^T/sqrt(D) + bias[k]) (ScalarE; bias -1e4 masks invalid
                                           key rows -> exp underflows to 0)
  rowsum[q] += ones.T @ W^T               (PE, M=1, bf16, PSUM accumulate)
  O^T[d, q] += V_tile.T @ W^T             (PE, bf16, PSUM accumulate)
then O^T * (1/rowsum) (GPSIMD partition-broadcast + DVE), PE-transpose
back to [q, d], evict via ScalarE, DMA out. bf16 is used for the
probability/value matmuls because fp32r accumulating matmuls
(start=False) measured ~15x slower on hardware.
"""

import math

import ml_dtypes
import numpy as np

import concourse.bass as bass
import concourse.mybir as mybir
import concourse.tile as tile
from concourse import bacc
from concourse.bass_utils import run_bass_kernel_spmd
from concourse.masks import make_identity

B, LQ, LK, D = 16, 2048, 2048, 128
N_CORES = 8
QB = 512          # query block (one unit) = QB rows of Q
N_SLOTS = (LQ // QB) * B // N_CORES   # 8 slots per core
KT = 128          # key tile
F32 = mybir.dt.float32
F32R = mybir.dt.float32r
BF16 = mybir.dt.bfloat16
FP16 = mybir.dt.float16
SCALE = 1.0 / math.sqrt(D)


def _plan(valid_lens):
    """Assign 64 (batch, qblock) units to an 8x8 (slot, core) grid.

    Returns (slot_units, slot_ntiles, masked_from) where
      slot_units[s][c] = (batch, qblock) handled by core c in slot s
      slot_ntiles[s]   = k-tiles processed in slot s (max over cores)
      masked_from[s]   = first k-tile index needing a mask multiply
    """
    vl = np.asarray(valid_lens).astype(np.int64)
    ktiles = np.maximum(1, np.ceil(vl / KT).astype(np.int64))
    units = [(int(b), j) for b in range(B) for j in range(LQ // QB)]
    units.sort(key=lambda u: -ktiles[u[0]])
    slot_units, slot_ntiles, masked_from = [], [], []
    for s in range(N_SLOTS):
        chunk = units[s * N_CORES:(s + 1) * N_CORES]
        slot_units.append(chunk)
        slot_ntiles.append(int(max(ktiles[b] for b, _ in chunk)))
        # tile t is fully valid for all cores iff t < min(floor(vl/KT));
        # from there on a multiplicative mask is required on some core.
        masked_from.append(int(min(vl[b] // KT for b, _ in chunk)))
    return slot_units, slot_ntiles, masked_from


def _pack_inputs(queries, keys, values, valid_lens, slot_units, slot_ntiles):
    """Build per-core packed input arrays (host-side numpy)."""
    vl = np.asarray(valid_lens).astype(np.int64)
    qt = np.ascontiguousarray(np.transpose(queries, (0, 2, 1)))   # [B, D, LQ]
    kt = np.ascontiguousarray(np.transpose(keys, (0, 2, 1)))      # [B, D, LK]
    u_total = sum(slot_ntiles)
    in_maps = []
    for c in range(N_CORES):
        qt_p = np.zeros((N_SLOTS, D, QB), np.float16)
        kt_p = np.zeros((u_total, D, KT), np.float16)
        v_p = np.zeros((u_total, KT, D), np.float16)
        mask_p = np.zeros((KT, u_total), np.float32)
        off = 0
        for s in range(N_SLOTS):
            b, j = slot_units[s][c]
            qt_p[s] = qt[b, :, j * QB:(j + 1) * QB]
            n_valid = int(min(slot_ntiles[s], math.ceil(vl[b] / KT)))
            for t in range(n_valid):
                k0 = t * KT
                kt_p[off + t] = kt[b, :, k0:k0 + KT]
                v_p[off + t] = values[b, k0:k0 + KT, :]
                nv = int(min(KT, vl[b] - k0))
                mask_p[:nv, off + t] = 1.0
            off += slot_ntiles[s]
        bias_p = (mask_p - 1.0) * 10000.0
        in_maps.append({
            "qt": qt_p, "kt": kt_p, "v": v_p, "mask": bias_p,
            "ones": np.ones((KT, 1), np.float16),
        })
    return in_maps, u_total


def build_kernel(slot_ntiles, masked_from, u_total, reps=1,
                 do_exp=True, do_rs=True, do_pv=True, do_epi=True):
    nc = bacc.Bacc(None, target_bir_lowering=False, debug=True)
    qt_d = nc.dram_tensor("qt", [N_SLOTS, D, QB], FP16, kind="ExternalInput")
    kt_d = nc.dram_tensor("kt", [u_total, D, KT], FP16, kind="ExternalInput")
    v_d = nc.dram_tensor("v", [u_total, KT, D], FP16, kind="ExternalInput")
    mask_d = nc.dram_tensor("mask", [KT, u_total], F32, kind="ExternalInput")
    ones_d = nc.dram_tensor("ones", [KT, 1], FP16, kind="ExternalInput")
    out_d = nc.dram_tensor("out", [N_SLOTS, QB, D], F32, kind="ExternalOutput")

    G = 2  # k-tiles per exp group (PSUM banks: st 2x2 + ot 2 + rs 1 + to 1 = 8)

    with tile.TileContext(nc) as tc:
        with (
            tc.tile_pool(name="const", bufs=1) as const,
            tc.tile_pool(name="wt_pool", bufs=4) as wt_pool,
            tc.tile_pool(name="ws_pool", bufs=3) as ws_pool,
            tc.tile_pool(name="onorm_pool", bufs=2) as onorm_pool,
            tc.tile_pool(name="ostage_pool", bufs=2) as ostage_pool,
            tc.tile_pool(name="recip_pool", bufs=2) as recip_pool,
            tc.tile_pool(name="bc_pool", bufs=2) as bc_pool,
            tc.tile_pool(name="st_psum", bufs=2, space="PSUM") as st_psum,
            tc.tile_pool(name="ot_psum", bufs=2, space="PSUM") as ot_psum,
            tc.tile_pool(name="rs_psum", bufs=1, space="PSUM") as rs_psum,
            tc.tile_pool(name="to_psum", bufs=1, space="PSUM") as to_psum,
        ):
            identity = const.tile([128, 128], FP16)
            make_identity(nc, identity)
            ones = const.tile([128, 1], FP16)
            nc.sync.dma_start(out=ones, in_=ones_d[:, :])
            qt_all = const.tile([128, N_SLOTS, QB], FP16)
            for s0 in range(0, N_SLOTS, 2):
                nc.sync.dma_start(
                    out=qt_all[:, s0:s0 + 2, :],
                    in_=qt_d[s0:s0 + 2].rearrange("s d q -> d s q"),
                )
            mask_all = const.tile([128, u_total], F32)
            nc.sync.dma_start(out=mask_all, in_=mask_d[:, :])
            kt_all = const.tile([128, u_total, KT], FP16)
            v_all = const.tile([128, u_total, D], FP16)
            bnds = [0, 2, 6] + [
                round(6 + i * (u_total - 6) / 6) for i in range(1, 7)]
            for lo, hi in zip(bnds[:-1], bnds[1:]):
                nc.sync.dma_start(
                    out=kt_all[:, lo:hi, :],
                    in_=kt_d[lo:hi].rearrange("u d k -> d u k"))
                nc.sync.dma_start(
                    out=v_all[:, lo:hi, :],
                    in_=v_d[lo:hi].rearrange("u k d -> k u d"))

            # reps>1 repeats computation for timing isolation (same output)
            pending_epi = []
            for _rep in range(reps):
                off = 0
                for s in range(len(slot_ntiles)):
                    u_s = slot_ntiles[s]
                    ot = ot_psum.tile([128, QB], F32)          # O^T accum [d, q]
                    rs = rs_psum.tile([1, QB], F32)            # rowsum [1, q]
                    qt_s = qt_all[:, s, :]
                    # merged exps below masked_from; per-tile exps with an
                    # additive mask bias (-1e4 -> exp underflows to 0) after
                    groups, t0 = [], 0
                    while t0 < u_s:
                        if t0 < masked_from[s]:
                            gsz = min(G, masked_from[s] - t0, u_s - t0)
                        else:
                            gsz = 1
                        groups.append((t0, gsz))
                        t0 += gsz
                    rs_pend = None
                    rs_emitted = 0
                    for gi, (g, gsz) in enumerate(groups):
                        st = st_psum.tile([128, G, QB], F32)   # S^T group
                        wt = wt_pool.tile([128, G, QB], FP16)   # exp(S^T) group
                        for tt in range(gsz):
                            t = g + tt
                            nc.tensor.matmul(
                                st[:, tt, :], kt_all[:, off + t, :], qt_s,
                                start=True, stop=True,
                            )
                        bias = (0.0 if g < masked_from[s]
                                else mask_all[:, off + g:off + g + 1])
                        nc.scalar.activation(
                            wt[:, :gsz, :], st[:, :gsz, :],
                            mybir.ActivationFunctionType.Exp, scale=SCALE,
                            bias=bias,
                        )
                        if g == 0 and pending_epi:
                            # previous slot's epilogue goes here, behind this
                            # slot's first QK group, so the in-order PE queue
                            # never stalls on the normalize chain
                            pending_epi.pop(0)()
                        if do_rs:
                            # rowsum is linear in the k-tiles: DVE sum-tree
                            # (pair tiles, then pair groups) so one PE rowsum
                            # matmul covers up to 4 k-tiles
                            if gsz == 2:
                                ws = ws_pool.tile([128, QB], FP16)
                                nc.vector.tensor_add(
                                    ws, wt[:, 0, :], wt[:, 1, :])
                                rs_src = ws
                            else:
                                rs_src = wt[:, 0, :]
                            if rs_pend is None and gi < len(groups) - 1:
                                rs_pend = rs_src
                            else:
                                if rs_pend is not None:
                                    ws2 = ws_pool.tile([128, QB], FP16)
                                    nc.vector.tensor_add(
                                        ws2, rs_pend, rs_src)
                                    rs_src = ws2
                                    rs_pend = None
                                nc.tensor.matmul(
                                    rs, ones, rs_src,
                                    start=(rs_emitted == 0),
                                    stop=(gi == len(groups) - 1),
                                )
                                rs_emitted += 1
                        for tt in range(gsz):
                            t = g + tt
                            wt_t = wt[:, tt, :]
                            if do_pv == "noaccum":
                                nc.tensor.matmul(
                                    ot, v_all[:, off + t, :], wt_t,
                                    start=True, stop=True,
                                )
                            elif do_pv:
                                nc.tensor.matmul(
                                    ot, v_all[:, off + t, :], wt_t,
                                    start=(t == 0), stop=(t == u_s - 1),
                                )
                    off += u_s
                    if not do_epi:
                        continue

                    def _epilogue(s=s, ot=ot, rs=rs):
                        recip = recip_pool.tile([1, QB], F32)
                        nc.vector.reciprocal(recip, rs)
                        recip_bc = bc_pool.tile([128, QB], F32)
                        nc.gpsimd.partition_broadcast(recip_bc, recip)
                        onorm = onorm_pool.tile([128, QB], FP16)
                        nc.vector.tensor_mul(onorm, ot, recip_bc)
                        to = to_psum.tile([128, QB], FP16)
                        for j in range(4):
                            nc.tensor.transpose(
                                to[:, j * 128:(j + 1) * 128],
                                onorm[:, j * 128:(j + 1) * 128],
                                identity,
                            )
                        ostage = ostage_pool.tile([128, 4, 128], F32)
                        nc.vector.tensor_copy(ostage, to)
                        nc.sync.dma_start(
                            out=out_d[s].rearrange(
                                "(sub p) d -> p sub d", p=128),
                            in_=ostage,
                        )
                    pending_epi.append(_epilogue)
            for fn in pending_epi:
                fn()
    nc.finalize()
    return nc


def kernel(queries, keys, values, valid_lens):
    queries = np.ascontiguousarray(np.asarray(queries, dtype=np.float32))
    keys = np.ascontiguousarray(np.asarray(keys, dtype=np.float32))
    values = np.ascontiguousarray(np.asarray(values, dtype=np.float32))
    assert queries.shape == (B, LQ, D), queries.shape
    assert keys.shape == (B, LK, D), keys.shape
    assert values.shape == (B, LK, D), values.shape

    slot_units, slot_ntiles, masked_from = _plan(valid_lens)
    in_maps, u_total = _pack_inputs(
        queries, keys, values, valid_lens, slot_units, slot_ntiles)
    nc = build_kernel(slot_ntiles, masked_from, u_total)
    res = None
    last_exc = None
    for attempt in range(3):
        try:
            res = run_bass_kernel_spmd(nc, in_maps, list(range(N_CORES)))
            break
        except Exception as exc:  # transient NRT/axon failures
            last_exc = exc
            try:
                import jax
                jax.clear_caches()
            except Exception:
                pass
    if res is None:
        raise last_exc

    out = np.empty((B, LQ, D), np.float32)
    for c in range(N_CORES):
        o = res.results[c]["out"]
        for s in range(N_SLOTS):
            b, j = slot_units[s][c]
            out[b, j * QB:(j + 1) * QB, :] = o[s]
    return out



# revision 29
# speedup vs baseline: 1.4158x; 1.2030x over previous
"""Masked dot-product attention on 8 Trainium2 NeuronCores (Bass/Tile).

Problem: B=16, LQ=LK=2048, D=128 fp32; per-batch key valid_lens mask.
Sharding: 64 (batch, 512-query-block) units bin-packed into 8 slots x 8
cores by per-batch valid k-tile count, so every core runs an identical
(SPMD) program while skipping masked-out key tiles entirely.

Per unit, on device (scores kept transposed so no probability transpose
is ever needed):
  S^T[k, q]  = KT_tile.T @ QT_block       (fp32r matmuls, N=512, full rate)
  W^T        = exp(S^T/sqrt(D) + bias[k]) (ScalarE; bias -1e4 masks invalid
                                           key rows -> exp underflows to 0)
  rowsum[q] += ones.T @ W^T               (PE, M=1, bf16, PSUM accumulate)
  O^T[d, q] += V_tile.T @ W^T             (PE, bf16, PSUM accumulate)
then O^T * (1/rowsum) (GPSIMD partition-broadcast + DVE), PE-transpose
back to [q, d], evict via ScalarE, DMA out. bf16 is used for the
probability/value matmuls because fp32r accumulating matmuls
(start=False) measured ~15x slower on hardware.
"""

import math

import ml_dtypes
import numpy as np

import concourse.bass as bass
import concourse.mybir as mybir
import concourse.tile as tile
from concourse import bacc
from concourse.bass_utils import run_bass_kernel_spmd
from concourse.masks import make_identity

B, LQ, LK, D = 16, 2048, 2048, 128
N_CORES = 8
QB = 512          # query block (one unit) = QB rows of Q
N_SLOTS = (LQ // QB) * B // N_CORES   # 8 slots per core
KT = 128          # key tile
F32 = mybir.dt.float32
F32R = mybir.dt.float32r
BF16 = mybir.dt.bfloat16
FP16 = mybir.dt.float16
SCALE = 1.0 / math.sqrt(D)


def _plan(valid_lens):
    """Assign 64 (batch, qblock) units to an 8x8 (slot, core) grid.

    Returns (slot_units, slot_ntiles, masked_from) where
      slot_units[s][c] = (batch, qblock) handled by core c in slot s
      slot_ntiles[s]   = k-tiles processed in slot s (max over cores)
      masked_from[s]   = first k-tile index needing a mask multiply
    """
    vl = np.asarray(valid_lens).astype(np.int64)
    ktiles = np.maximum(1, np.ceil(vl / KT).astype(np.int64))
    units = [(int(b), j) for b in range(B) for j in range(LQ // QB)]
    units.sort(key=lambda u: -ktiles[u[0]])
    slot_units, slot_ntiles, masked_from = [], [], []
    for s in range(N_SLOTS):
        chunk = units[s * N_CORES:(s + 1) * N_CORES]
        slot_units.append(chunk)
        slot_ntiles.append(int(max(ktiles[b] for b, _ in chunk)))
        # tile t is fully valid for all cores iff t < min(floor(vl/KT));
        # from there on a multiplicative mask is required on some core.
        masked_from.append(int(min(vl[b] // KT for b, _ in chunk)))
    return slot_units, slot_ntiles, masked_from


def _pack_inputs(queries, keys, values, valid_lens, slot_units, slot_ntiles):
    """Build per-core packed input arrays (host-side numpy)."""
    vl = np.asarray(valid_lens).astype(np.int64)
    qt = np.ascontiguousarray(np.transpose(queries, (0, 2, 1)))   # [B, D, LQ]
    kt = np.ascontiguousarray(np.transpose(keys, (0, 2, 1)))      # [B, D, LK]
    u_total = sum(slot_ntiles)
    in_maps = []
    for c in range(N_CORES):
        qt_p = np.zeros((N_SLOTS, D, QB), np.float16)
        kt_p = np.zeros((u_total, D, KT), np.float16)
        v_p = np.zeros((u_total, KT, D), np.float16)
        mask_p = np.zeros((KT, u_total), np.float32)
        off = 0
        for s in range(N_SLOTS):
            b, j = slot_units[s][c]
            qt_p[s] = qt[b, :, j * QB:(j + 1) * QB]
            n_valid = int(min(slot_ntiles[s], math.ceil(vl[b] / KT)))
            for t in range(n_valid):
                k0 = t * KT
                kt_p[off + t] = kt[b, :, k0:k0 + KT]
                v_p[off + t] = values[b, k0:k0 + KT, :]
                nv = int(min(KT, vl[b] - k0))
                mask_p[:nv, off + t] = 1.0
            off += slot_ntiles[s]
        bias_p = (mask_p - 1.0) * 10000.0
        in_maps.append({
            "qt": qt_p, "kt": kt_p, "v": v_p, "mask": bias_p,
            "ones": np.ones((KT, 1), np.float16),
        })
    return in_maps, u_total


def build_kernel(slot_ntiles, masked_from, u_total, reps=1,
                 do_exp=True, do_rs=True, do_pv=True, do_epi=True):
    nc = bacc.Bacc(None, target_bir_lowering=False, debug=True)
    qt_d = nc.dram_tensor("qt", [N_SLOTS, D, QB], FP16, kind="ExternalInput")
    kt_d = nc.dram_tensor("kt", [u_total, D, KT], FP16, kind="ExternalInput")
    v_d = nc.dram_tensor("v", [u_total, KT, D], FP16, kind="ExternalInput")
    mask_d = nc.dram_tensor("mask", [KT, u_total], F32, kind="ExternalInput")
    ones_d = nc.dram_tensor("ones", [KT, 1], FP16, kind="ExternalInput")
    out_d = nc.dram_tensor("out", [N_SLOTS, QB, D], F32, kind="ExternalOutput")

    G = 2  # k-tiles per exp group (PSUM banks: st 2x2 + ot 2 + rs 1 + to 1 = 8)

    with tile.TileContext(nc) as tc:
        with (
            tc.tile_pool(name="const", bufs=1) as const,
            tc.tile_pool(name="wt_pool", bufs=5) as wt_pool,
            tc.tile_pool(name="ws_pool", bufs=5) as ws_pool,
            tc.tile_pool(name="onorm_pool", bufs=2) as onorm_pool,
            tc.tile_pool(name="ostage_pool", bufs=2) as ostage_pool,
            tc.tile_pool(name="recip_pool", bufs=2) as recip_pool,
            tc.tile_pool(name="bc_pool", bufs=2) as bc_pool,
            tc.tile_pool(name="st_psum", bufs=2, space="PSUM") as st_psum,
            tc.tile_pool(name="ot_psum", bufs=2, space="PSUM") as ot_psum,
            tc.tile_pool(name="rs_psum", bufs=1, space="PSUM") as rs_psum,
            tc.tile_pool(name="to_psum", bufs=1, space="PSUM") as to_psum,
        ):
            identity = const.tile([128, 128], FP16)
            make_identity(nc, identity)
            ones = const.tile([128, 1], FP16)
            nc.sync.dma_start(out=ones, in_=ones_d[:, :])
            qt_all = const.tile([128, N_SLOTS, QB], FP16)
            for s0 in range(0, N_SLOTS, 2):
                nc.sync.dma_start(
                    out=qt_all[:, s0:s0 + 2, :],
                    in_=qt_d[s0:s0 + 2].rearrange("s d q -> d s q"),
                )
            mask_all = const.tile([128, u_total], F32)
            nc.sync.dma_start(out=mask_all, in_=mask_d[:, :])
            kt_all = const.tile([128, u_total, KT], FP16)
            v_all = const.tile([128, u_total, D], FP16)
            bnds = [0, 2, 6] + [
                round(6 + i * (u_total - 6) / 6) for i in range(1, 7)]
            for lo, hi in zip(bnds[:-1], bnds[1:]):
                nc.sync.dma_start(
                    out=kt_all[:, lo:hi, :],
                    in_=kt_d[lo:hi].rearrange("u d k -> d u k"))
                nc.sync.dma_start(
                    out=v_all[:, lo:hi, :],
                    in_=v_d[lo:hi].rearrange("u k d -> k u d"))

            # reps>1 repeats computation for timing isolation (same output)
            pending_epi = []
            for _rep in range(reps):
                off = 0
                for s in range(len(slot_ntiles)):
                    u_s = slot_ntiles[s]
                    ot = ot_psum.tile([128, QB], F32)          # O^T accum [d, q]
                    rs = rs_psum.tile([1, QB], F32)            # rowsum [1, q]
                    qt_s = qt_all[:, s, :]
                    # merged exps below masked_from; per-tile exps with an
                    # additive mask bias (-1e4 -> exp underflows to 0) after
                    groups, t0 = [], 0
                    while t0 < u_s:
                        if t0 < masked_from[s]:
                            gsz = min(G, masked_from[s] - t0, u_s - t0)
                        else:
                            gsz = 1
                        groups.append((t0, gsz))
                        t0 += gsz
                    rs_pend = None
                    rs_emitted = 0
                    for gi, (g, gsz) in enumerate(groups):
                        st = st_psum.tile([128, G, QB], F32)   # S^T group
                        wt = wt_pool.tile([128, G, QB], FP16)   # exp(S^T) group
                        for tt in range(gsz):
                            t = g + tt
                            nc.tensor.matmul(
                                st[:, tt, :], kt_all[:, off + t, :], qt_s,
                                start=True, stop=True,
                            )
                        bias = (0.0 if g < masked_from[s]
                                else mask_all[:, off + g:off + g + 1])
                        nc.scalar.activation(
                            wt[:, :gsz, :], st[:, :gsz, :],
                            mybir.ActivationFunctionType.Exp, scale=SCALE,
                            bias=bias,
                        )
                        if g == 0 and pending_epi:
                            # previous slot's epilogue goes here, behind this
                            # slot's first QK group, so the in-order PE queue
                            # never stalls on the normalize chain
                            pending_epi.pop(0)()
                        if do_rs:
                            # rowsum is linear in the k-tiles: DVE sum-tree
                            # (pair tiles, then pair groups) so one PE rowsum
                            # matmul covers up to 4 k-tiles
                            if gsz == 2:
                                ws = ws_pool.tile([128, QB], FP16)
                                nc.vector.tensor_add(
                                    ws, wt[:, 0, :], wt[:, 1, :])
                                rs_src = ws
                            else:
                                rs_src = wt[:, 0, :]
                            if rs_pend is None and gi < len(groups) - 1:
                                rs_pend = rs_src
                            else:
                                if rs_pend is not None:
                                    ws2 = ws_pool.tile([128, QB], FP16)
                                    nc.vector.tensor_add(
                                        ws2, rs_pend, rs_src)
                                    rs_src = ws2
                                    rs_pend = None
                                nc.tensor.matmul(
                                    rs, ones, rs_src,
                                    start=(rs_emitted == 0),
                                    stop=(gi == len(groups) - 1),
                                )
                                rs_emitted += 1
                        for tt in range(gsz):
                            t = g + tt
                            wt_t = wt[:, tt, :]
                            if do_pv == "noaccum":
                                nc.tensor.matmul(
                                    ot, v_all[:, off + t, :], wt_t,
                                    start=True, stop=True,
                                )
                            elif do_pv:
                                nc.tensor.matmul(
                                    ot, v_all[:, off + t, :], wt_t,
                                    start=(t == 0), stop=(t == u_s - 1),
                                )
                    off += u_s
                    if not do_epi:
                        continue

                    def _epilogue(s=s, ot=ot, rs=rs):
                        recip = recip_pool.tile([1, QB], F32)
                        nc.vector.reciprocal(recip, rs)
                        recip_bc = bc_pool.tile([128, QB], F32)
                        nc.gpsimd.partition_broadcast(recip_bc, recip)
                        onorm = onorm_pool.tile([128, QB], FP16)
                        nc.vector.tensor_mul(onorm, ot, recip_bc)
                        to = to_psum.tile([128, QB], FP16)
                        for j in range(4):
                            nc.tensor.transpose(
                                to[:, j * 128:(j + 1) * 128],
                                onorm[:, j * 128:(j + 1) * 128],
                                identity,
                            )
                        ostage = ostage_pool.tile([128, 4, 128], F32)
                        nc.vector.tensor_copy(ostage, to)
                        nc.sync.dma_start(
                            out=out_d[s].rearrange(
                                "(sub p) d -> p sub d", p=128),
                            in_=ostage,
                        )
                    pending_epi.append(_epilogue)
            for fn in pending_epi:
                fn()
    nc.finalize()
    return nc


def kernel(queries, keys, values, valid_lens):
    queries = np.ascontiguousarray(np.asarray(queries, dtype=np.float32))
    keys = np.ascontiguousarray(np.asarray(keys, dtype=np.float32))
    values = np.ascontiguousarray(np.asarray(values, dtype=np.float32))
    assert queries.shape == (B, LQ, D), queries.shape
    assert keys.shape == (B, LK, D), keys.shape
    assert values.shape == (B, LK, D), values.shape

    slot_units, slot_ntiles, masked_from = _plan(valid_lens)
    in_maps, u_total = _pack_inputs(
        queries, keys, values, valid_lens, slot_units, slot_ntiles)
    nc = build_kernel(slot_ntiles, masked_from, u_total)
    res = None
    last_exc = None
    for attempt in range(3):
        try:
            res = run_bass_kernel_spmd(nc, in_maps, list(range(N_CORES)))
            break
        except Exception as exc:  # transient NRT/axon failures
            last_exc = exc
            try:
                import jax
                jax.clear_caches()
            except Exception:
                pass
    if res is None:
        raise last_exc

    out = np.empty((B, LQ, D), np.float32)
    for c in range(N_CORES):
        o = res.results[c]["out"]
        for s in range(N_SLOTS):
            b, j = slot_units[s][c]
            out[b, j * QB:(j + 1) * QB, :] = o[s]
    return out

